# revision 1
# baseline (speedup 1.0000x reference)
"""Bass/Trainium2 kernel for GQA decode attention (fused K-projection form).

Reference computation (per problem spec):
  x = x_pre[:, -1, :]                               # [16, 4096]
  xq = (x @ wq.T) -> [b, 32, 128]
  qt[b,h,:] = xq[b,h,:] @ wk[kv(h)*128:+128, :]     # [b, 32, 4096]
  scores = qt . x_pre / sqrt(128)                   # [b, 32, 2048]
  attn = softmax_t(scores)
  ctx[b,h,:] = sum_t attn[b,h,t] * x_pre[b,t,:]     # [b, 32, 4096]  (lazy-V)
  out[b,h,d] = sum_D ctx[b,h,D] * wv[kv(h)*128+d,D] # [b, 32, 128]
  y = out.flat @ wo.T                               # [16, 4096]

Sharding (8 cores): batch-parallel attention (2 batches/core) +
head-parallel projections (4 heads = 1 kv head/core), exchanged with two
AllToAll collectives. wo is column-sharded (contraction dim); host sums
the 8 partial y outputs.
"""

import math

import numpy as np

import concourse.bass as bass
import concourse.mybir as mybir
import concourse.tile as tile
from concourse import bacc
from concourse.bass_utils import run_bass_kernel_spmd
from concourse.masks import make_identity
from concourse.tile import add_dep_helper

F32 = mybir.dt.float32
NC = 8
BSZ = 16
SEQ = 2048
DIM = 4096
NH = 32
HD = 128
B_LOC = 2        # batches per core
HL = 4           # local heads per core
NT = SEQ // 128  # 16 t-tiles per batch
NDC = DIM // 128 # 32 D-chunks
SCALE = 1.0 / math.sqrt(HD)


def build_program(trace_label="", debug=False, nocc=False, skel=False):
    nc = bacc.Bacc("TRN2", target_bir_lowering=False, debug=False)

    xp = nc.dram_tensor("xp", [B_LOC, SEQ, DIM], F32, kind="ExternalInput")
    xl = nc.dram_tensor("xl", [BSZ, DIM], F32, kind="ExternalInput")
    wq = nc.dram_tensor("wq", [HL * HD, DIM], F32, kind="ExternalInput")
    wk = nc.dram_tensor("wk", [HD, DIM], F32, kind="ExternalInput")
    wv = nc.dram_tensor("wv", [HD, DIM], F32, kind="ExternalInput")
    wo = nc.dram_tensor("wo", [DIM, HL * HD], F32, kind="ExternalInput")
    y = nc.dram_tensor("y", [BSZ, DIM], F32, kind="ExternalOutput")
    if debug:
        dbg_q = nc.dram_tensor("dbg_q", [B_LOC * NH, DIM], F32,
                               kind="ExternalOutput")
        dbg_ctx = nc.dram_tensor("dbg_ctx", [BSZ * HL, DIM], F32,
                                 kind="ExternalOutput")
        dbg_sc = nc.dram_tensor("dbg_sc", [B_LOC * NH, 128], F32,
                                kind="ExternalOutput")
        dbg_xq = nc.dram_tensor("dbg_xq", [BSZ, HL * HD], F32,
                                kind="ExternalOutput")
        dbg_qs = nc.dram_tensor("dbg_qs", [BSZ, DIM], F32,
                                kind="ExternalOutput")

    rg = [list(range(NC))]

    with tile.TileContext(nc) as tc:
        with (
            tc.tile_pool(name="persist", bufs=1) as pers,
            tc.tile_pool(name="dram", bufs=1, space="DRAM") as dram,
        ):
            ident = pers.tile([128, 128], F32)
            make_identity(nc, ident)

            # DRAM exchange buffers
            a2a1_in = dram.tile([NC * B_LOC * HL, DIM], F32)   # [64, 4096]
            a2a1_out = dram.tile([NC * B_LOC * HL, DIM], F32)
            a2a2_in = dram.tile([NC * B_LOC * HL, DIM], F32)
            a2a2_out = dram.tile([NC * B_LOC * HL, DIM], F32)

            stage_dmas1 = []
            stage_dmas2 = []
            # ---------------- Phase 1: q-tilde for local heads, all batches
            with (
                tc.tile_pool(name="p1", bufs=2) as p1,
                tc.tile_pool(name="p1w", bufs=1) as p1w,
                tc.tile_pool(name="p1ps", bufs=2, space="PSUM") as p1ps,
            ):
                xl_sb = p1w.tile([BSZ, DIM], F32)
                nc.sync.dma_start(out=xl_sb, in_=xl[:, :])
                wk_sb = p1w.tile([HD, DIM], F32)
                nc.sync.dma_start(out=wk_sb, in_=wk[:, :])

                # xT: [128 D x 16 b] per D-chunk
                xT = p1w.tile([128, NDC * BSZ], F32)
                for c in range(NDC):
                    tp = p1ps.tile([128, BSZ], F32, tag="tp1")
                    nc.tensor.transpose(tp, xl_sb[:, c * 128:(c + 1) * 128],
                                        ident[0:BSZ, 0:BSZ])
                    nc.vector.tensor_copy(out=xT[:, c * BSZ:(c + 1) * BSZ], in_=tp)

                # wqT: per D-chunk c: [128 D x 512 hd]
                wqT = p1w.tile([128, NDC * HL * HD], F32)
                for m in range(HL):
                    wq_sb = p1.tile([128, DIM], F32, tag="wqnat")
                    nc.sync.dma_start(out=wq_sb, in_=wq[m * 128:(m + 1) * 128, :])
                    for c in range(NDC):
                        tp = p1ps.tile([128, 128], F32, tag="tp1")
                        nc.tensor.transpose(tp, wq_sb[:, c * 128:(c + 1) * 128],
                                            ident)
                        nc.vector.tensor_copy(
                            out=wqT[:, c * 512 + m * 128: c * 512 + (m + 1) * 128],
                            in_=tp)

                # xq = x @ wq_slice.T : accumulate over D-chunks -> [16 b, 512 hd]
                xq_ps = p1ps.tile([BSZ, HL * HD], F32, tag="xq")
                for c in range(NDC):
                    nc.tensor.matmul(xq_ps, xT[:, c * BSZ:(c + 1) * BSZ],
                                     wqT[:, c * 512:(c + 1) * 512],
                                     start=(c == 0), stop=(c == NDC - 1))
                xq_sb = p1w.tile([BSZ, HL * HD], F32)
                nc.vector.tensor_copy(out=xq_sb, in_=xq_ps)
                if debug:
                    nc.sync.dma_start(out=dbg_xq[:, :], in_=xq_sb)

                # xqT: [128 d x 16 b] per local head
                xqT = p1w.tile([128, HL * BSZ], F32)
                for m in range(HL):
                    tp = p1ps.tile([128, BSZ], F32, tag="tp1")
                    nc.tensor.transpose(tp, xq_sb[:, m * 128:(m + 1) * 128],
                                        ident[0:BSZ, 0:BSZ])
                    nc.vector.tensor_copy(out=xqT[:, m * BSZ:(m + 1) * BSZ], in_=tp)

                # qt[h] = xq[:,h,:] @ wk_kv  (scaled) -> staged [64, 4096]
                # row layout = h_loc*16 + b
                for m in range(HL):
                    qstage = p1.tile([BSZ, DIM], F32, tag="qstage")
                    for j in range(8):
                        q_ps = p1ps.tile([BSZ, 512], F32, tag="qps")
                        nc.tensor.matmul(q_ps, xqT[:, m * BSZ:(m + 1) * BSZ],
                                         wk_sb[:, j * 512:(j + 1) * 512],
                                         start=True, stop=True)
                        nc.scalar.mul(
                            out=qstage[:, j * 512:(j + 1) * 512],
                            in_=q_ps, mul=SCALE)
                    d = nc.sync.dma_start(
                        out=a2a1_in.rearrange("(r b h) d -> h r b d",
                                              r=NC, b=B_LOC)[m],
                        in_=qstage)
                    stage_dmas1.append(d)
                    if debug and m == 0:
                        nc.sync.dma_start(out=dbg_qs[:, :], in_=qstage)


            if not nocc:
                cc1 = nc.gpsimd.collective_compute(
                    "AllToAll", mybir.AluOpType.bypass,
                    ins=[a2a1_in.opt()], outs=[a2a1_out.opt()], replica_groups=rg)
                for d in stage_dmas1:
                    add_dep_helper(cc1.ins, d.ins, reason="a2a1 input ready")

            # qT per local batch: [128 D x 32 h] per D-chunk
            # a2a1_out row = src_r*8 + b_loc*4 + h_loc ; head = 4*src_r + h_loc
            qT = [pers.tile([128, NDC * NH], F32, tag=f"qT{b}", name=f"qT{b}")
                  for b in range(B_LOC)]
            with (
                tc.tile_pool(name="qnat", bufs=2) as qnatp,
                tc.tile_pool(name="qnps", bufs=2, space="PSUM") as qnps,
            ):
                for b in range(B_LOC):
                    qnat = qnatp.tile([NH, DIM], F32, tag="qnat")
                    d = nc.sync.dma_start(
                        out=qnat,
                        in_=a2a1_out.rearrange("(r b h) d -> b r h d",
                                               r=NC, b=B_LOC)[b])
                    if not nocc:
                        add_dep_helper(d.ins, cc1.ins, reason="a2a1 done")
                    if debug:
                        nc.sync.dma_start(out=dbg_q[b * NH:(b + 1) * NH, :],
                                          in_=qnat)
                    for c in range(NDC):
                        tp = qnps.tile([128, NH], F32, tag="tpq")
                        nc.tensor.transpose(tp, qnat[:, c * 128:(c + 1) * 128],
                                            ident[0:NH, 0:NH])
                        nc.vector.tensor_copy(
                            out=qT[b][:, c * NH:(c + 1) * NH], in_=tp)

            # ---------------- Phase 2: streaming attention per local batch
            with (
                tc.tile_pool(name="xpool", bufs=6) as xpool,
                tc.tile_pool(name="xtpool", bufs=1) as xtpool,
                tc.tile_pool(name="attn", bufs=3) as apool,
                tc.tile_pool(name="small", bufs=2) as smallp,
                tc.tile_pool(name="ctx_sb", bufs=1) as ctxsbp,
                tc.tile_pool(name="tps", bufs=3, space="PSUM") as tps,
                tc.tile_pool(name="scps", bufs=2, space="PSUM") as scps,
                tc.tile_pool(name="ctxps", bufs=1, space="PSUM") as ctxps,
            ):
                for b in range(B_LOC):
                    ctx_ps = ctxps.tile([128, 1024], F32, tag="ctx")
                    sums = smallp.tile([NH, 4], F32, tag="sums")
                    for ch in range(4):
                        xts = []
                        for tt in range(4):
                            t = ch * 4 + tt
                            x_sb = xpool.tile([128, DIM], F32, tag="x",
                                              name=f"x{b}_{t}")
                            nc.sync.dma_start(
                                out=x_sb, in_=xp[b, t * 128:(t + 1) * 128, :])
                            xts.append(x_sb)
                        xtt = xtpool.tile([128, NDC * 512], F32, tag="xt")
                        xv = xtt.rearrange("p (c t) -> p c t", c=NDC)
                        for tt in range(4):
                            for cg in range(NDC // 4):
                                tp = tps.tile([128, 512], F32, tag="tp2")
                                for cc in range(4):
                                    c = cg * 4 + cc
                                    nc.tensor.transpose(
                                        tp[:, cc * 128:(cc + 1) * 128],
                                        xts[tt][:, c * 128:(c + 1) * 128], ident)
                                nc.vector.tensor_copy(
                                    out=xv[:, cg * 4:(cg + 1) * 4,
                                           tt * 128:(tt + 1) * 128],
                                    in_=tp.rearrange("p (a q) -> p a q", a=4))
                        if skel:
                            continue
                        sc_ps = scps.tile([NH, 512], F32, tag="sc")
                        for c in range(NDC):
                            nc.tensor.matmul(sc_ps,
                                             qT[b][:, c * NH:(c + 1) * NH],
                                             xtt[:, c * 512:(c + 1) * 512],
                                             start=(c == 0), stop=(c == NDC - 1))
                        attn_sb = apool.tile([NH, 512], F32, tag="attn")
                        nc.scalar.activation(out=attn_sb, in_=sc_ps,
                                             func=mybir.ActivationFunctionType.Exp,
                                             accum_out=sums[:, ch:ch + 1])
                        at_ps = tps.tile([128, 512], F32, tag="tp2")
                        for tt in range(4):
                            nc.tensor.transpose(
                                at_ps[:, tt * NH:(tt + 1) * NH],
                                attn_sb[:, tt * 128:(tt + 1) * 128],
                                ident[0:NH, 0:NH])
                        at_sb = apool.tile([128, 4 * NH], F32, tag="attnT")
                        nc.vector.tensor_copy(out=at_sb, in_=at_ps[:, 0:4 * NH])
                        for tt in range(4):
                            for g in range(4):
                                for jj in (g, g + 4):
                                    nc.tensor.matmul(
                                        ctx_ps[g * 32:(g + 1) * 32,
                                               (jj // 4) * 512:(jj // 4 + 1) * 512],
                                        at_sb[:, tt * NH:(tt + 1) * NH],
                                        xts[tt][:, jj * 512:(jj + 1) * 512],
                                        start=(ch == 0 and tt == 0),
                                        stop=(ch == 3 and tt == 3),
                                        tile_position=(0, g * 32))
                    # finalize batch: 1/rowsum, scale, stage for exchange
                    ssum = smallp.tile([NH, 1], F32, tag="ssum")
                    nc.vector.reduce_sum(out=ssum, in_=sums,
                                         axis=mybir.AxisListType.X)
                    rsum = smallp.tile([128, 1], F32, tag="rsum")
                    nc.vector.reciprocal(out=rsum[0:NH], in_=ssum)
                    for g in range(1, 4):
                        nc.vector.tensor_copy(out=rsum[g * 32:(g + 1) * 32],
                                              in_=rsum[0:NH])
                    ctx_sb = ctxsbp.tile([NH, DIM], F32, tag="ctxsb")
                    for j in range(8):
                        g = j % 4
                        nc.vector.tensor_scalar_mul(
                            ctx_sb[:, j * 512:(j + 1) * 512],
                            ctx_ps[g * 32:(g + 1) * 32,
                                   (j // 4) * 512:(j // 4 + 1) * 512],
                            rsum[g * 32:(g + 1) * 32, 0:1])
                    # ship: dest rank r gets heads 4r..4r+3 ; row r*8 + b*4 + h
                    d = nc.sync.dma_start(
                        out=a2a2_in.rearrange("(r b h) d -> r b h d",
                                              r=NC, b=B_LOC)[:, b],
                        in_=ctx_sb)
                    stage_dmas2.append(d)

            if not nocc:
                cc2 = nc.gpsimd.collective_compute(
                    "AllToAll", mybir.AluOpType.bypass,
                    ins=[a2a2_in.opt()], outs=[a2a2_out.opt()], replica_groups=rg)
                for d in stage_dmas2:
                    add_dep_helper(cc2.ins, d.ins, reason="a2a2 input ready")

            # ---------------- Phase 3: output projection (local heads, all b)
            with (
                tc.tile_pool(name="p3", bufs=2) as p3,
                tc.tile_pool(name="p3w", bufs=1) as p3w,
                tc.tile_pool(name="p3ps", bufs=2, space="PSUM") as p3ps,
            ):
                # a2a2_out row = src_r*8 + b_loc*4 + h = b*4 + h  (b=2*src_r+b_loc)
                ctxg = p3w.tile([BSZ * HL, DIM], F32)
                d = nc.sync.dma_start(out=ctxg, in_=a2a2_out[:, :])
                if not nocc:
                    add_dep_helper(d.ins, cc2.ins, reason="a2a2 done")
                if debug:
                    nc.sync.dma_start(out=dbg_ctx[:, :], in_=ctxg)
                ctxgT = p3w.tile([128, NDC * BSZ * HL], F32)
                for c in range(NDC):
                    tp = p3ps.tile([128, 128], F32, tag="tp3")
                    nc.tensor.transpose(tp[:, 0:BSZ * HL],
                                        ctxg[:, c * 128:(c + 1) * 128],
                                        ident[0:BSZ * HL, 0:BSZ * HL])
                    nc.vector.tensor_copy(out=ctxgT[:, c * 64:(c + 1) * 64],
                                          in_=tp[:, 0:64])

                wv_sb = p3w.tile([HD, DIM], F32)
                nc.sync.dma_start(out=wv_sb, in_=wv[:, :])
                wvT = p3w.tile([128, DIM], F32)
                for c in range(NDC):
                    tp = p3ps.tile([128, 128], F32, tag="tp3")
                    nc.tensor.transpose(tp, wv_sb[:, c * 128:(c + 1) * 128], ident)
                    nc.vector.tensor_copy(out=wvT[:, c * 128:(c + 1) * 128], in_=tp)

                # out[b*4+h, d] accumulation over D-chunks
                op_ps = p3ps.tile([BSZ * HL, HD], F32, tag="op")
                for c in range(NDC):
                    nc.tensor.matmul(op_ps, ctxgT[:, c * 64:(c + 1) * 64],
                                     wvT[:, c * 128:(c + 1) * 128],
                                     start=(c == 0), stop=(c == NDC - 1))
                op_sb = p3w.tile([BSZ * HL, HD], F32)
                nc.vector.tensor_copy(out=op_sb, in_=op_ps)
                # outT [128 d x 64 (b*4+h)]
                otp = p3ps.tile([128, 128], F32, tag="tp3")
                nc.tensor.transpose(otp[:, 0:64], op_sb,
                                    ident[0:BSZ * HL, 0:BSZ * HL])
                outT = p3w.tile([128, BSZ * HL], F32)
                nc.vector.tensor_copy(out=outT, in_=otp[:, 0:64])

                # woT: per h_rel m: [128 d x 4096 j]
                woT = p3w.tile([128, HL * DIM], F32)
                for jt in range(NDC):
                    wo_sb = p3.tile([128, HL * HD], F32, tag="wonat")
                    nc.sync.dma_start(out=wo_sb, in_=wo[jt * 128:(jt + 1) * 128, :])
                    for m in range(HL):
                        tp = p3ps.tile([128, 128], F32, tag="tp3")
                        nc.tensor.transpose(tp, wo_sb[:, m * 128:(m + 1) * 128],
                                            ident)
                        nc.vector.tensor_copy(
                            out=woT[:, m * DIM + jt * 128: m * DIM + (jt + 1) * 128],
                            in_=tp)

                # y partial [16 b, 4096 j]
                y_sb = p3w.tile([BSZ, DIM], F32)
                oT = outT.rearrange("p (b h) -> p h b", h=HL)
                for jc in range(8):
                    y_ps = p3ps.tile([BSZ, 512], F32, tag="yps")
                    for m in range(HL):
                        nc.tensor.matmul(y_ps, oT[:, m, :],
                                         woT[:, m * DIM + jc * 512:
                                             m * DIM + (jc + 1) * 512],
                                         start=(m == 0), stop=(m == HL - 1))
                    nc.vector.tensor_copy(out=y_sb[:, jc * 512:(jc + 1) * 512],
                                          in_=y_ps)
                nc.sync.dma_start(out=y[:, :], in_=y_sb)

    nc.finalize()
    return nc


_PROGRAM_CACHE = {}


def kernel(x_pre, wq, wk, wv, wo, _trace=False, _tmpdir=None):
    x_pre = np.ascontiguousarray(np.asarray(x_pre, dtype=np.float32))
    wq = np.asarray(wq, dtype=np.float32)
    wk = np.asarray(wk, dtype=np.float32)
    wv = np.asarray(wv, dtype=np.float32)
    wo = np.asarray(wo, dtype=np.float32)

    if "nc" not in _PROGRAM_CACHE:
        _PROGRAM_CACHE["nc"] = build_program()
    nc = _PROGRAM_CACHE["nc"]

    xl = np.ascontiguousarray(x_pre[:, -1, :])
    in_maps = []
    for i in range(NC):
        in_maps.append({
            "xp": np.ascontiguousarray(x_pre[2 * i:2 * i + 2]),
            "xl": xl,
            "wq": np.ascontiguousarray(wq[512 * i:512 * (i + 1), :]),
            "wk": np.ascontiguousarray(wk[128 * i:128 * (i + 1), :]),
            "wv": np.ascontiguousarray(wv[128 * i:128 * (i + 1), :]),
            "wo": np.ascontiguousarray(wo[:, 512 * i:512 * (i + 1)]),
        })

    kwargs = {}
    if _trace:
        kwargs = dict(trace=True, trace_cores=[0])
    if _tmpdir is not None:
        kwargs["tmpdir"] = _tmpdir
    res = run_bass_kernel_spmd(nc, in_maps, core_ids=list(range(NC)), **kwargs)
    y = np.zeros((BSZ, DIM), np.float32)
    for i in range(NC):
        y += res.results[i]["y"]
    if _trace:
        print("HW exec time:", res.exec_time_ns, "ns")
    return y.reshape(BSZ, 1, DIM)



# revision 17
# speedup vs baseline: 2.3506x; 2.3506x over previous
"""Bass/Trainium2 kernel for GQA decode attention (fused K-projection form).

Reference computation (per problem spec):
  x = x_pre[:, -1, :]                               # [16, 4096]
  xq = (x @ wq.T) -> [b, 32, 128]
  qt[b,h,:] = xq[b,h,:] @ wk[kv(h)*128:+128, :]     # [b, 32, 4096]
  scores = qt . x_pre / sqrt(128)                   # [b, 32, 2048]
  attn = softmax_t(scores)
  ctx[b,h,:] = sum_t attn[b,h,t] * x_pre[b,t,:]     # [b, 32, 4096]  (lazy-V)
  out[b,h,d] = sum_D ctx[b,h,D] * wv[kv(h)*128+d,D] # [b, 32, 128]
  y = out.flat @ wo.T                               # [16, 4096]

Sharding (8 cores): batch-parallel attention (2 batches/core) +
head-parallel projections (4 heads = 1 kv head/core).  All on-chip compute
in bf16 (fp32 PSUM accumulation); HBM loads cast f32->bf16 during the DMA
(SWDGE).  Two AllToAlls redistribute qt (head->batch) and ctx
(batch->head), each split per local batch so the exchange overlaps
compute.  wo is column-sharded; host sums the 8 partial y outputs.
"""

import math

import numpy as np

import concourse.mybir as mybir
import concourse.tile as tile
from concourse import bacc
from concourse.bass_utils import run_bass_kernel_spmd
from concourse.masks import make_identity
from concourse.tile import add_dep_helper

F32 = mybir.dt.float32
BF16 = mybir.dt.bfloat16
NC = 8
BSZ = 16
SEQ = 2048
DIM = 4096
NH = 32
HD = 128
NKV = 8
B_LOC = 2        # batches per core
HL = 4           # local heads per core
NT = SEQ // 128  # 16 t-tiles per batch
NCH = 8          # 256-token chunks per batch
NDC = DIM // 128 # 32 D-chunks
SCALE = 1.0 / math.sqrt(HD)
EXP = mybir.ActivationFunctionType.Exp


def build_program(debug=False):
    nc = bacc.Bacc("TRN2", target_bir_lowering=False, debug=False)

    def act_copy(out, in_):
        return nc.scalar.mul(out=out, in_=in_, mul=1.0)

    xp = nc.dram_tensor("xp", [B_LOC, SEQ, DIM], F32, kind="ExternalInput")
    xl = nc.dram_tensor("xl", [BSZ, DIM], F32, kind="ExternalInput")
    wq = nc.dram_tensor("wq", [HL * HD, DIM], F32, kind="ExternalInput")
    wk = nc.dram_tensor("wk", [HD, DIM], F32, kind="ExternalInput")
    wv = nc.dram_tensor("wv", [HD, DIM], F32, kind="ExternalInput")
    wo = nc.dram_tensor("wo", [DIM, HL * HD], F32, kind="ExternalInput")
    y = nc.dram_tensor("y", [BSZ, DIM], F32, kind="ExternalOutput")
    if debug:
        dbg_qt = nc.dram_tensor("dbg_qt", [B_LOC * NH, DIM], BF16,
                                kind="ExternalOutput")
        dbg_xq = nc.dram_tensor("dbg_xq", [BSZ, HL * HD], BF16,
                                kind="ExternalOutput")
        dbg_qts = nc.dram_tensor("dbg_qts", [BSZ * HL, DIM], BF16,
                                 kind="ExternalOutput")
        dbg_ctx = nc.dram_tensor("dbg_ctx", [B_LOC * NH, DIM], BF16,
                                 kind="ExternalOutput")

    rg = [list(range(NC))]

    from contextlib import ExitStack
    with tile.TileContext(nc) as tc:
        with ExitStack() as es:
            pers = es.enter_context(tc.tile_pool(name="persist", bufs=1))
            dram = es.enter_context(tc.tile_pool(name="dram", bufs=1, space="DRAM"))
            p1 = es.enter_context(tc.tile_pool(name="p1", bufs=2))
            p1w = es.enter_context(tc.tile_pool(name="p1w", bufs=1))
            gpool = es.enter_context(tc.tile_pool(name="gpool", bufs=2))
            pw = es.enter_context(tc.tile_pool(name="pw", bufs=2))
            xpool = es.enter_context(tc.tile_pool(name="xpool", bufs=4))
            xtpool = es.enter_context(tc.tile_pool(name="xtpool", bufs=2))
            apool = es.enter_context(tc.tile_pool(name="attn", bufs=2))
            smallp = es.enter_context(tc.tile_pool(name="small", bufs=2))
            ctxsbp = es.enter_context(tc.tile_pool(name="ctx_sbp", bufs=1))
            p3 = es.enter_context(tc.tile_pool(name="p3", bufs=1))
            tps = es.enter_context(tc.tile_pool(name="tps", bufs=3, space="PSUM"))
            fps = es.enter_context(tc.tile_pool(name="fps", bufs=2, space="PSUM"))
            scps = es.enter_context(tc.tile_pool(name="scps", bufs=1, space="PSUM"))
            ctxps = es.enter_context(tc.tile_pool(name="ctxps", bufs=1, space="PSUM"))
            ident = pers.tile([128, 128], BF16)
            make_identity(nc, ident)

            # DRAM exchange buffers (bf16)
            a2a1_in = [dram.tile([NC * HL, DIM], BF16, name=f"a1i{b}")
                       for b in range(B_LOC)]
            a2a1_out = [dram.tile([NC * HL, DIM], BF16, name=f"a1o{b}")
                        for b in range(B_LOC)]
            a2a2_in = [dram.tile([NC * HL, DIM], BF16, name=f"a2i{b}")
                       for b in range(B_LOC)]
            a2a2_out = [dram.tile([NC * HL, DIM], BF16, name=f"a2o{b}")
                        for b in range(B_LOC)]

            # persistent SBUF: per-batch qt^T [128 D, (c, head)] and
            # projection weights
            qT = [pers.tile([128, NDC * NH], BF16, name=f"qT{b}")
                  for b in range(B_LOC)]
            wvT = pers.tile([128, DIM], BF16)
            woT = pers.tile([128, HL * DIM], BF16)

            # ---------------- Phase 1: qt for local heads, all batches
            xl_sb = p1w.tile([BSZ, DIM], BF16, name="xl_sb")
            nc.gpsimd.dma_start(out=xl_sb, in_=xl[:, :])
            wk_sb = p1w.tile([HD, DIM], BF16, name="wk_sb")
            nc.gpsimd.dma_start(out=wk_sb, in_=wk[:, :])

            # xlT: [128 D x 16 b] per D-chunk
            xlT = p1w.tile([128, NDC * BSZ], BF16, name="xlT")
            for q in range(4):
                tp = tps.tile([128, 512], BF16, tag="tp")
                for i in range(8):
                    c = q * 8 + i
                    nc.tensor.transpose(
                        tp[:, i * BSZ:(i + 1) * BSZ],
                        xl_sb[:, c * 128:(c + 1) * 128],
                        ident[0:BSZ, 0:BSZ])
                nc.vector.tensor_copy(
                    out=xlT[:, q * 8 * BSZ:(q + 1) * 8 * BSZ],
                    in_=tp[:, 0:8 * BSZ])

            # xq = x @ wq_slice.T, one 128-row block of wq at a time
            xq_f = fps.tile([64, 512], F32, tag="f")
            xq_ps = xq_f[0:BSZ, :]
            for m in range(HL):
                wq_m = p1.tile([128, DIM], BF16, tag="wqm")
                nc.gpsimd.dma_start(out=wq_m,
                                    in_=wq[m * 128:(m + 1) * 128, :])
                wqT_m = p1.tile([128, DIM], BF16, tag="wqTm")
                for cg in range(8):
                    tp = tps.tile([128, 512], BF16, tag="tp")
                    for i in range(4):
                        c = cg * 4 + i
                        nc.tensor.transpose(tp[:, i * 128:(i + 1) * 128],
                                            wq_m[:, c * 128:(c + 1) * 128],
                                            ident)
                    cp = (nc.vector.tensor_copy if cg % 2 == 0
                          else act_copy)
                    cp(out=wqT_m[:, cg * 512:(cg + 1) * 512], in_=tp)
                for c in range(NDC):
                    nc.tensor.matmul(xq_ps[:, m * 128:(m + 1) * 128],
                                     xlT[:, c * BSZ:(c + 1) * BSZ],
                                     wqT_m[:, c * 128:(c + 1) * 128],
                                     start=(c == 0), stop=(c == NDC - 1))
            # scale by 1/sqrt(HD) here (folds the score scaling)
            xq_sb = p1w.tile([BSZ, HL * HD], BF16, name="xq_sb")
            nc.scalar.mul(out=xq_sb, in_=xq_ps, mul=SCALE)

            # xqT: [128 d, (m, b)]
            xqT = p1w.tile([128, HL * BSZ], BF16, name="xqT")
            tpq_t = tps.tile([128, 512], BF16, tag="tp")
            tpq = tpq_t[:, 0:HL * BSZ]
            for m in range(HL):
                nc.tensor.transpose(
                    tpq[:, m * BSZ:(m + 1) * BSZ],
                    xq_sb[:, m * 128:(m + 1) * 128],
                    ident[0:BSZ, 0:BSZ])
            # reorder columns (m, r2, bl) -> (bl, r2, m) during the copy
            nc.vector.tensor_copy(
                out=xqT.rearrange("p (bl r2 m) -> p m r2 bl",
                                  bl=B_LOC, r2=NC, m=HL),
                in_=tpq.rearrange("p (m r2 bl) -> p m r2 bl",
                                  m=HL, r2=NC, bl=B_LOC))
            xq_bm = xqT

            # qt rows (b, m): [64, 4096] = xq_bm.T @ wk (natural layout!)
            qt_sb = p1w.tile([BSZ * HL, DIM], BF16, name="qt_sb")
            for j in range(8):
                q_ps = fps.tile([64, 512], F32, tag="f")
                nc.tensor.matmul(q_ps, xq_bm,
                                 wk_sb[:, j * 512:(j + 1) * 512],
                                 start=True, stop=True)
                cp = (nc.vector.tensor_copy if j % 2 == 0
                      else act_copy)
                cp(out=qt_sb[:, j * 512:(j + 1) * 512], in_=q_ps)

            if debug:
                nc.sync.dma_start(out=dbg_xq[:, :], in_=xq_sb)
                nc.sync.dma_start(out=dbg_qts[:, :], in_=qt_sb)

            # stage per b_loc: a2a1_in[bl] rows (dest r, m) - contiguous
            cc1 = []
            for bl in range(B_LOC):
                d = nc.sync.dma_start(
                    out=a2a1_in[bl][:, :],
                    in_=qt_sb[bl * 32:(bl + 1) * 32, :])
                cc = nc.gpsimd.collective_compute(
                    "AllToAll", mybir.AluOpType.bypass,
                    ins=[a2a1_in[bl].opt()], outs=[a2a1_out[bl].opt()],
                    replica_groups=rg)
                add_dep_helper(cc.ins, d.ins, reason="a2a1 input ready")
                cc1.append(cc)

            # ---------------- Phase 1b: unpack qt^T per local batch
            for b in range(B_LOC):
                qnat = gpool.tile([NH, DIM], BF16, tag="gbuf")
                d = nc.sync.dma_start(out=qnat, in_=a2a1_out[b][:, :])
                add_dep_helper(d.ins, cc1[b].ins, reason="a2a1 done")
                if debug:
                    nc.sync.dma_start(out=dbg_qt[b * NH:(b + 1) * NH, :],
                                      in_=qnat)
                for cg in range(8):
                    tp_t = tps.tile([128, 512], BF16, tag="tp")
                    tp = tp_t[:, 0:4 * NH]
                    for i in range(4):
                        c = cg * 4 + i
                        nc.tensor.transpose(
                            tp[:, i * NH:(i + 1) * NH],
                            qnat[:, c * 128:(c + 1) * 128],
                            ident[0:NH, 0:NH])
                    cp = (nc.vector.tensor_copy if cg % 2 == 0
                          else act_copy)
                    cp(out=qT[b][:, cg * 4 * NH:(cg + 1) * 4 * NH], in_=tp)

            # ---------------- Phase W: wv^T / wo^T prep (overlaps phase 2)
            wv_sb = p1w.tile([HD, DIM], BF16, name="wv_sb")
            nc.gpsimd.dma_start(out=wv_sb, in_=wv[:, :])
            for cg in range(8):
                tp = tps.tile([128, 512], BF16, tag="tp")
                for i in range(4):
                    c = cg * 4 + i
                    nc.tensor.transpose(tp[:, i * 128:(i + 1) * 128],
                                        wv_sb[:, c * 128:(c + 1) * 128],
                                        ident)
                cp = (nc.vector.tensor_copy if cg % 2 == 0
                      else act_copy)
                cp(out=wvT[:, cg * 512:(cg + 1) * 512], in_=tp)

            # wo^T streamed one 512-row group at a time
            for cg in range(8):
                wo_cg = pw.tile([128, 4 * HL * HD], BF16, tag="wocg")
                nc.gpsimd.dma_start(
                    out=wo_cg,
                    in_=wo[cg * 512:(cg + 1) * 512, :].rearrange(
                        "(c p) j -> p c j", p=128))
                for m in range(HL):
                    tp = tps.tile([128, 512], BF16, tag="tp")
                    for i in range(4):
                        nc.tensor.transpose(
                            tp[:, i * 128:(i + 1) * 128],
                            wo_cg[:, i * 512 + m * 128:i * 512 + (m + 1) * 128],
                            ident)
                    cp = (nc.vector.tensor_copy if m % 2 == 0
                          else act_copy)
                    cp(out=woT[:, m * DIM + cg * 512:m * DIM + (cg + 1) * 512],
                       in_=tp)

            # ---------------- Phase 2: streaming attention per local batch
            for b in range(B_LOC):
                ctx_ps = ctxps.tile([128, 1024], F32, tag="ctx")
                sums = smallp.tile([NH, NCH], F32, tag="sums")
                for ch in range(NCH):
                    xts = []
                    for tt in range(2):
                        t = ch * 2 + tt
                        x_sb = xpool.tile([128, DIM], BF16, tag="x",
                                          name=f"x{b}_{t}")
                        nc.gpsimd.dma_start(
                            out=x_sb, in_=xp[b, t * 128:(t + 1) * 128, :])
                        xts.append(x_sb)
                    sc_ps = scps.tile([NH, 256], F32, tag="sc")
                    for tt in range(2):
                        xtt = xtpool.tile([128, DIM], BF16, tag="xt")
                        for cg in range(8):
                            tp = tps.tile([128, 512], BF16, tag="tp")
                            for i in range(4):
                                c = cg * 4 + i
                                nc.tensor.transpose(
                                    tp[:, i * 128:(i + 1) * 128],
                                    xts[tt][:, c * 128:(c + 1) * 128],
                                    ident)
                            cp = (nc.vector.tensor_copy if cg % 3 != 2
                                  else act_copy)
                            cp(out=xtt[:, cg * 512:(cg + 1) * 512], in_=tp)
                        for c in range(NDC):
                            nc.tensor.matmul(
                                sc_ps[:, tt * 128:(tt + 1) * 128],
                                qT[b][:, c * NH:(c + 1) * NH],
                                xtt[:, c * 128:(c + 1) * 128],
                                start=(c == 0), stop=(c == NDC - 1))
                    attn_sb = apool.tile([NH, 256], BF16, tag="attn")
                    nc.scalar.activation(out=attn_sb, in_=sc_ps, func=EXP,
                                         accum_out=sums[:, ch:ch + 1])
                    at_ps = tps.tile([128, 512], BF16, tag="tp")
                    for tt in range(2):
                        nc.tensor.transpose(
                            at_ps[:, tt * NH:(tt + 1) * NH],
                            attn_sb[:, tt * 128:(tt + 1) * 128],
                            ident[0:NH, 0:NH])
                    at_sb = apool.tile([128, 2 * NH], BF16, tag="attnT")
                    nc.vector.tensor_copy(out=at_sb, in_=at_ps[:, 0:2 * NH])
                    # ctx accumulate: quadrant g holds D-chunks 2g, 2g+1
                    for tt in range(2):
                        for g in range(4):
                            for jj in range(2):
                                nc.tensor.matmul(
                                    ctx_ps[g * 32:(g + 1) * 32,
                                           jj * 512:(jj + 1) * 512],
                                    at_sb[:, tt * NH:(tt + 1) * NH],
                                    xts[tt][:, (2 * g + jj) * 512:
                                            (2 * g + jj + 1) * 512],
                                    start=(ch == 0 and tt == 0),
                                    stop=(ch == NCH - 1 and tt == 1),
                                    tile_position=(0, g * 32))
                # finalize batch: 1/rowsum, scale, stage for exchange
                ssum = smallp.tile([NH, 1], F32, tag="ssum")
                nc.vector.reduce_sum(out=ssum, in_=sums,
                                     axis=mybir.AxisListType.X)
                rsum = smallp.tile([128, 1], F32, tag="rsum")
                nc.vector.reciprocal(out=rsum[0:NH], in_=ssum)
                for g in range(1, 4):
                    nc.vector.tensor_copy(out=rsum[g * 32:(g + 1) * 32],
                                          in_=rsum[0:NH])
                ctx_sb = ctxsbp.tile([NH, DIM], BF16, tag="ctxsb")
                for j in range(8):
                    g = j // 2
                    src = ctx_ps[g * 32:(g + 1) * 32,
                                 (j % 2) * 512:(j % 2 + 1) * 512]
                    if j % 2 == 0:
                        nc.vector.tensor_scalar_mul(
                            ctx_sb[:, j * 512:(j + 1) * 512], src,
                            rsum[g * 32:(g + 1) * 32, 0:1])
                    else:
                        nc.scalar.mul(
                            out=ctx_sb[:, j * 512:(j + 1) * 512],
                            in_=src, mul=rsum[g * 32:(g + 1) * 32, 0:1])
                if debug:
                    nc.sync.dma_start(out=dbg_ctx[b * NH:(b + 1) * NH, :],
                                      in_=ctx_sb)
                # ship: dest rank r gets heads 4r..4r+3
                d = nc.sync.dma_start(out=a2a2_in[b][:, :], in_=ctx_sb)
                cc2 = nc.gpsimd.collective_compute(
                    "AllToAll", mybir.AluOpType.bypass,
                    ins=[a2a2_in[b].opt()], outs=[a2a2_out[b].opt()],
                    replica_groups=rg)
                add_dep_helper(cc2.ins, d.ins, reason="a2a2 input ready")

                # ---- phase 3 for this b_loc: wv projection + y partial
                ctxg = gpool.tile([NC * HL, DIM], BF16, tag="gbuf")
                d = nc.sync.dma_start(out=ctxg, in_=a2a2_out[b][:, :])
                add_dep_helper(d.ins, cc2.ins, reason="a2a2 done")
                ctxgT = p3.tile([128, NDC * NC * HL], BF16, tag="ctxgT")
                for cg in range(4):
                    tp_t = tps.tile([128, 512], BF16, tag="tp")
                    tp = tp_t[:, 0:8 * 32]
                    for i in range(8):
                        c = cg * 8 + i
                        nc.tensor.transpose(
                            tp[:, i * 32:(i + 1) * 32],
                            ctxg[:, c * 128:(c + 1) * 128],
                            ident[0:32, 0:32])
                    cp = (nc.vector.tensor_copy if cg % 2 == 0
                          else act_copy)
                    cp(out=ctxgT[:, cg * 256:(cg + 1) * 256], in_=tp)
                op_f = fps.tile([64, 512], F32, tag="f")
                op_ps = op_f[0:NC * HL, 0:HD]
                for c in range(NDC):
                    nc.tensor.matmul(op_ps, ctxgT[:, c * 32:(c + 1) * 32],
                                     wvT[:, c * 128:(c + 1) * 128],
                                     start=(c == 0), stop=(c == NDC - 1))
                op_sb = p3.tile([NC * HL, HD], BF16, tag="op_sb")
                nc.vector.tensor_copy(out=op_sb, in_=op_ps)
                otp_t = tps.tile([128, 512], BF16, tag="tp")
                otp = otp_t[:, 0:32]
                nc.tensor.transpose(otp, op_sb, ident[0:32, 0:32])
                outT = p3.tile([128, NC * HL], BF16, tag="outT")
                nc.vector.tensor_copy(out=outT, in_=otp)
                oT = outT.rearrange("p (c h) -> p h c", h=HL)
                y_sb = p3.tile([NC, DIM], BF16, tag="ysb")
                for jc in range(8):
                    y_f = fps.tile([64, 512], F32, tag="f")
                    y_ps = y_f[0:NC, :]
                    for m in range(HL):
                        nc.tensor.matmul(
                            y_ps, oT[:, m, :],
                            woT[:, m * DIM + jc * 512:m * DIM + (jc + 1) * 512],
                            start=(m == 0), stop=(m == HL - 1))
                    cp = (nc.vector.tensor_copy if jc % 2 == 0
                          else act_copy)
                    cp(out=y_sb[:, jc * 512:(jc + 1) * 512], in_=y_ps)
                nc.gpsimd.dma_start(out=y[b * NC:(b + 1) * NC, :], in_=y_sb)

    nc.finalize()
    return nc


_PROGRAM_CACHE = {}


def kernel(x_pre, wq, wk, wv, wo, _trace=False, _tmpdir=None, _debug=False):
    x_pre = np.ascontiguousarray(np.asarray(x_pre, dtype=np.float32))
    wq = np.asarray(wq, dtype=np.float32)
    wk = np.asarray(wk, dtype=np.float32)
    wv = np.asarray(wv, dtype=np.float32)
    wo = np.asarray(wo, dtype=np.float32)

    key = ("dbg" if _debug else "nc")
    if key not in _PROGRAM_CACHE:
        _PROGRAM_CACHE[key] = build_program(debug=_debug)
    nc = _PROGRAM_CACHE[key]

    xl = np.ascontiguousarray(x_pre[:, -1, :])
    in_maps = []
    for i in range(NC):
        in_maps.append({
            "xp": np.ascontiguousarray(x_pre[2 * i:2 * i + 2]),
            "xl": xl,
            "wq": np.ascontiguousarray(wq[512 * i:512 * (i + 1), :]),
            "wk": np.ascontiguousarray(wk[128 * i:128 * (i + 1), :]),
            "wv": np.ascontiguousarray(wv[128 * i:128 * (i + 1), :]),
            "wo": np.ascontiguousarray(wo[:, 512 * i:512 * (i + 1)]),
        })

    kwargs = {}
    if _trace:
        kwargs = dict(trace=True, trace_cores=[0])
    if _tmpdir is not None:
        kwargs["tmpdir"] = _tmpdir
    res = run_bass_kernel_spmd(nc, in_maps, core_ids=list(range(NC)), **kwargs)
    y = np.zeros((BSZ, DIM), np.float32)
    for i in range(NC):
        yi = res.results[i]["y"]
        # kernel y row r = 8*b_loc + c  ->  global batch 2*c + b_loc
        y[0::2] += yi[0:NC]
        y[1::2] += yi[NC:2 * NC]
    if _debug:
        _PROGRAM_CACHE["dbg_res"] = res
    if _trace:
        print("HW exec time:", res.exec_time_ns, "ns")
    return y.reshape(BSZ, 1, DIM)


# revision 19
# speedup vs baseline: 2.3820x; 1.0134x over previous
"""Bass/Trainium2 kernel for GQA decode attention (fused K-projection form).

Reference computation (per problem spec):
  x = x_pre[:, -1, :]                               # [16, 4096]
  xq = (x @ wq.T) -> [b, 32, 128]
  qt[b,h,:] = xq[b,h,:] @ wk[kv(h)*128:+128, :]     # [b, 32, 4096]
  scores = qt . x_pre / sqrt(128)                   # [b, 32, 2048]
  attn = softmax_t(scores)
  ctx[b,h,:] = sum_t attn[b,h,t] * x_pre[b,t,:]     # [b, 32, 4096]  (lazy-V)
  out[b,h,d] = sum_D ctx[b,h,D] * wv[kv(h)*128+d,D] # [b, 32, 128]
  y = out.flat @ wo.T                               # [16, 4096]

Sharding (8 cores): batch-parallel attention (2 batches/core) +
head-parallel projections (4 heads = 1 kv head/core).  All on-chip compute
in bf16 (fp32 PSUM accumulation); HBM loads cast f32->bf16 during the DMA
(SWDGE).  Two AllToAlls redistribute qt (head->batch) and ctx
(batch->head), each split per local batch so the exchange overlaps
compute.  wo is column-sharded; host sums the 8 partial y outputs.

The per-core program is laid out as a flat software pipeline over the 16
(batch, 256-token-chunk) attention steps so every engine's in-order
instruction stream matches expected data-arrival order (engine wait
queues are only 4 deep - a burst of >4 not-yet-satisfiable instructions
stalls the whole engine stream).
"""

import math
from contextlib import ExitStack

import numpy as np

import concourse.mybir as mybir
import concourse.tile as tile
from concourse import bacc
from concourse.bass_utils import run_bass_kernel_spmd
from concourse.masks import make_identity
from concourse.tile import add_dep_helper

F32 = mybir.dt.float32
BF16 = mybir.dt.bfloat16
NC = 8
BSZ = 16
SEQ = 2048
DIM = 4096
NH = 32
HD = 128
NKV = 8
B_LOC = 2        # batches per core
HL = 4           # local heads per core
NT = SEQ // 128  # 16 t-tiles per batch
NCH = 8          # 256-token chunks per batch
NK = B_LOC * NCH # 16 flat chunks
NDC = DIM // 128 # 32 D-chunks
SCALE = 1.0 / math.sqrt(HD)
EXP = mybir.ActivationFunctionType.Exp


def build_program(debug=False):
    nc = bacc.Bacc("TRN2", target_bir_lowering=False, debug=False)

    def act_copy(out, in_):
        return nc.scalar.mul(out=out, in_=in_, mul=1.0)

    xp = nc.dram_tensor("xp", [B_LOC, SEQ, DIM], F32, kind="ExternalInput")
    xl = nc.dram_tensor("xl", [BSZ, DIM], F32, kind="ExternalInput")
    wq = nc.dram_tensor("wq", [HL * HD, DIM], F32, kind="ExternalInput")
    wk = nc.dram_tensor("wk", [HD, DIM], F32, kind="ExternalInput")
    wv = nc.dram_tensor("wv", [HD, DIM], F32, kind="ExternalInput")
    wo = nc.dram_tensor("wo", [DIM, HL * HD], F32, kind="ExternalInput")
    y = nc.dram_tensor("y", [BSZ, DIM], F32, kind="ExternalOutput")
    if debug:
        dbg_qt = nc.dram_tensor("dbg_qt", [B_LOC * NH, DIM], BF16,
                                kind="ExternalOutput")
        dbg_ctx = nc.dram_tensor("dbg_ctx", [B_LOC * NH, DIM], BF16,
                                 kind="ExternalOutput")

    rg = [list(range(NC))]

    with tile.TileContext(nc) as tc:
        with ExitStack() as es:
            pers = es.enter_context(tc.tile_pool(name="persist", bufs=1))
            dram = es.enter_context(
                tc.tile_pool(name="dram", bufs=1, space="DRAM"))
            p1 = es.enter_context(tc.tile_pool(name="p1", bufs=2))
            p1w = es.enter_context(tc.tile_pool(name="p1w", bufs=1))
            gpool = es.enter_context(tc.tile_pool(name="gpool", bufs=1))
            pw = es.enter_context(tc.tile_pool(name="pw", bufs=2))
            xpool = es.enter_context(tc.tile_pool(name="xpool", bufs=4))
            xtpool = es.enter_context(tc.tile_pool(name="xtpool", bufs=2))
            apool = es.enter_context(tc.tile_pool(name="attn", bufs=2))
            smallp = es.enter_context(tc.tile_pool(name="small", bufs=2))
            ctxsbp = es.enter_context(tc.tile_pool(name="ctx_sbp", bufs=1))
            p3 = es.enter_context(tc.tile_pool(name="p3", bufs=1))
            # PSUM budget (8 banks): tp x3 + sc x3 + ctx(2 banks)
            tps = es.enter_context(
                tc.tile_pool(name="tps", bufs=3, space="PSUM"))
            scps = es.enter_context(
                tc.tile_pool(name="scps", bufs=3, space="PSUM"))
            ctxps = es.enter_context(
                tc.tile_pool(name="ctxps", bufs=1, space="PSUM"))

            ident = pers.tile([128, 128], BF16)
            make_identity(nc, ident)

            # DRAM exchange buffers (bf16)
            a2a1_in = [dram.tile([NC * HL, DIM], BF16, name=f"a1i{b}")
                       for b in range(B_LOC)]
            a2a1_out = [dram.tile([NC * HL, DIM], BF16, name=f"a1o{b}")
                        for b in range(B_LOC)]
            a2a2_in = [dram.tile([NC * HL, DIM], BF16, name=f"a2i{b}")
                       for b in range(B_LOC)]
            a2a2_out = [dram.tile([NC * HL, DIM], BF16, name=f"a2o{b}")
                        for b in range(B_LOC)]

            qT = [pers.tile([128, NDC * NH], BF16, name=f"qT{b}")
                  for b in range(B_LOC)]
            wvT = pers.tile([128, DIM], BF16)
            woT = pers.tile([128, HL * DIM], BF16)

            # ---------------- x-tile DMA prefetch machinery
            x_tiles = {}

            def issue_x(k):
                """DMA the two 128-token tiles of flat chunk k."""
                if k >= NK:
                    return
                b, ch = divmod(k, NCH)
                for tt in range(2):
                    t = ch * 2 + tt
                    x_sb = xpool.tile([128, DIM], BF16, tag="x",
                                      name=f"x{b}_{t}")
                    nc.gpsimd.dma_start(
                        out=x_sb, in_=xp[b, t * 128:(t + 1) * 128, :])
                    x_tiles[(b, t)] = x_sb

            # ---------------- Phase 1 + interleaved DMA prologue
            xl_sb = p1w.tile([BSZ, DIM], BF16, name="xl_sb")
            nc.gpsimd.dma_start(out=xl_sb, in_=xl[:, :])
            issue_x(0)

            # xlT: [128 D x 16 b] per D-chunk
            xlT = p1w.tile([128, NDC * BSZ], BF16, name="xlT")
            for q in range(4):
                tp = tps.tile([128, 512], BF16, tag="tp")
                for i in range(8):
                    c = q * 8 + i
                    nc.tensor.transpose(
                        tp[:, i * BSZ:(i + 1) * BSZ],
                        xl_sb[:, c * 128:(c + 1) * 128],
                        ident[0:BSZ, 0:BSZ])
                nc.vector.tensor_copy(
                    out=xlT[:, q * 8 * BSZ:(q + 1) * 8 * BSZ],
                    in_=tp[:, 0:8 * BSZ])

            # xq = x @ wq_slice.T, one 128-row block of wq at a time
            xq_f = ctxps.tile([128, 1024], F32, tag="ctx", name="xq_f")
            xq_ps = xq_f[0:BSZ, 0:HL * HD]
            for m in range(HL):
                wq_m = p1.tile([128, DIM], BF16, tag="wqm", bufs=1)
                nc.gpsimd.dma_start(out=wq_m,
                                    in_=wq[m * 128:(m + 1) * 128, :])
                wqT_m = p1.tile([128, DIM], BF16, tag="wqTm")
                for cg in range(8):
                    tp = tps.tile([128, 512], BF16, tag="tp")
                    for i in range(4):
                        c = cg * 4 + i
                        nc.tensor.transpose(tp[:, i * 128:(i + 1) * 128],
                                            wq_m[:, c * 128:(c + 1) * 128],
                                            ident)
                    cp = (nc.vector.tensor_copy if cg % 2 == 0
                          else act_copy)
                    cp(out=wqT_m[:, cg * 512:(cg + 1) * 512], in_=tp)
                for c in range(NDC):
                    nc.tensor.matmul(xq_ps[:, m * 128:(m + 1) * 128],
                                     xlT[:, c * BSZ:(c + 1) * BSZ],
                                     wqT_m[:, c * 128:(c + 1) * 128],
                                     start=(c == 0), stop=(c == NDC - 1))
            wk_sb = p1w.tile([HD, DIM], BF16, name="wk_sb")
            nc.gpsimd.dma_start(out=wk_sb, in_=wk[:, :])
            issue_x(1)

            # scale by 1/sqrt(HD) here (folds the score scaling)
            xq_sb = p1w.tile([BSZ, HL * HD], BF16, name="xq_sb")
            nc.scalar.mul(out=xq_sb, in_=xq_ps, mul=SCALE)

            # xqT columns ordered (bl, r2, m)  [source cols (m, r2, bl)]
            xqT = p1w.tile([128, HL * BSZ], BF16, name="xqT")
            tpq_t = tps.tile([128, 512], BF16, tag="tp")
            tpq = tpq_t[:, 0:HL * BSZ]
            for m in range(HL):
                nc.tensor.transpose(
                    tpq[:, m * BSZ:(m + 1) * BSZ],
                    xq_sb[:, m * 128:(m + 1) * 128],
                    ident[0:BSZ, 0:BSZ])
            nc.vector.tensor_copy(
                out=xqT.rearrange("p (bl r2 m) -> p m r2 bl",
                                  bl=B_LOC, r2=NC, m=HL),
                in_=tpq.rearrange("p (m r2 bl) -> p m r2 bl",
                                  m=HL, r2=NC, bl=B_LOC))

            # qt rows (bl, r2, m): [64, 4096] = xqT.T @ wk (natural layout!)
            qt_sb = p1w.tile([BSZ * HL, DIM], BF16, name="qt_sb")
            for j in range(8):
                q_f = scps.tile([64, 512], F32, tag="sc", name=f"q_f{j}")
                nc.tensor.matmul(q_f, xqT,
                                 wk_sb[:, j * 512:(j + 1) * 512],
                                 start=True, stop=True)
                cp = (nc.vector.tensor_copy if j % 2 == 0
                      else act_copy)
                cp(out=qt_sb[:, j * 512:(j + 1) * 512], in_=q_f)

            # stage per b_loc (contiguous row slices) + collectives
            cc1 = []
            for bl in range(B_LOC):
                d = nc.sync.dma_start(out=a2a1_in[bl][:, :],
                                      in_=qt_sb[bl * 32:(bl + 1) * 32, :])
                cc = nc.gpsimd.collective_compute(
                    "AllToAll", mybir.AluOpType.bypass,
                    ins=[a2a1_in[bl].opt()], outs=[a2a1_out[bl].opt()],
                    replica_groups=rg)
                add_dep_helper(cc.ins, d.ins, reason="a2a1 input ready")
                cc1.append(cc)

            # more DMA prologue on the Pool queue
            wv_sb = p1w.tile([HD, DIM], BF16, name="wv_sb")
            nc.gpsimd.dma_start(out=wv_sb, in_=wv[:, :])
            issue_x(2)
            wo_list = []
            for cg in range(8):
                wo_cg = pw.tile([128, 4 * HL * HD], BF16, tag="wocg",
                                name=f"wo{cg}", bufs=1)
                nc.gpsimd.dma_start(
                    out=wo_cg,
                    in_=wo[cg * 512:(cg + 1) * 512, :].rearrange(
                        "(c p) j -> p c j", p=128))
                wo_list.append(wo_cg)

            # ---------------- pipeline helper stages
            def transpose_chunk(k):
                """PE-transpose chunk k's two tiles into an xtt slot."""
                b, ch = divmod(k, NCH)
                xtt = xtpool.tile([128, NDC * 256], BF16, tag="xt",
                                  name=f"xt{k}")
                xv = xtt.rearrange("p (c t) -> p c t", c=NDC)
                for tt in range(2):
                    x_sb = x_tiles[(b, ch * 2 + tt)]
                    for cg in range(8):
                        tp = tps.tile([128, 512], BF16, tag="tp")
                        for i in range(4):
                            c = cg * 4 + i
                            nc.tensor.transpose(
                                tp[:, i * 128:(i + 1) * 128],
                                x_sb[:, c * 128:(c + 1) * 128],
                                ident)
                        cp = (nc.vector.tensor_copy if cg % 3 != 2
                              else act_copy)
                        cp(out=xv[:, cg * 4:(cg + 1) * 4,
                                  tt * 128:(tt + 1) * 128],
                           in_=tp.rearrange("p (a t) -> p a t", a=4))
                return xtt

            def wv_transposes():
                for cg in range(8):
                    tp = tps.tile([128, 512], BF16, tag="tp")
                    for i in range(4):
                        c = cg * 4 + i
                        nc.tensor.transpose(tp[:, i * 128:(i + 1) * 128],
                                            wv_sb[:, c * 128:(c + 1) * 128],
                                            ident)
                    cp = (nc.vector.tensor_copy if cg % 2 == 0
                          else act_copy)
                    cp(out=wvT[:, cg * 512:(cg + 1) * 512], in_=tp)

            def wo_transposes(cg):
                wo_cg = wo_list[cg]
                for m in range(HL):
                    tp = tps.tile([128, 512], BF16, tag="tp")
                    for i in range(4):
                        nc.tensor.transpose(
                            tp[:, i * 128:(i + 1) * 128],
                            wo_cg[:, i * 512 + m * 128:
                                  i * 512 + (m + 1) * 128],
                            ident)
                    cp = (nc.vector.tensor_copy if m % 2 == 0
                          else act_copy)
                    cp(out=woT[:, m * DIM + cg * 512:
                               m * DIM + (cg + 1) * 512],
                       in_=tp)

            def qnat_unpack(b):
                qnat = gpool.tile([NH, DIM], BF16, tag="gbuf",
                                  name=f"qnat{b}")
                d = nc.sync.dma_start(out=qnat, in_=a2a1_out[b][:, :])
                add_dep_helper(d.ins, cc1[b].ins, reason="a2a1 done")
                if debug:
                    nc.sync.dma_start(out=dbg_qt[b * NH:(b + 1) * NH, :],
                                      in_=qnat)
                for cg in range(8):
                    tp_t = tps.tile([128, 512], BF16, tag="tp")
                    tp = tp_t[:, 0:4 * NH]
                    for i in range(4):
                        c = cg * 4 + i
                        nc.tensor.transpose(
                            tp[:, i * NH:(i + 1) * NH],
                            qnat[:, c * 128:(c + 1) * 128],
                            ident[0:NH, 0:NH])
                    cp = (nc.vector.tensor_copy if cg % 2 == 0
                          else act_copy)
                    cp(out=qT[b][:, cg * 4 * NH:(cg + 1) * 4 * NH], in_=tp)

            ctx_ps = [None, None]
            sums = [None, None]

            def attend_chunk(k, xtt):
                b, ch = divmod(k, NCH)
                if ch == 0:
                    ctx_ps[b] = ctxps.tile([128, 1024], F32, tag="ctx",
                                           name=f"ctx{b}")
                    sums[b] = smallp.tile([NH, NCH], F32, tag="sums",
                                          name=f"sums{b}")
                sc_ps_t = scps.tile([64, 512], F32, tag="sc",
                                    name=f"sc{k}")
                sc_ps = sc_ps_t[0:NH, 0:256]
                for c in range(NDC):
                    nc.tensor.matmul(
                        sc_ps, qT[b][:, c * NH:(c + 1) * NH],
                        xtt[:, c * 256:(c + 1) * 256],
                        start=(c == 0), stop=(c == NDC - 1))
                attn_sb = apool.tile([NH, 256], BF16, tag="attn")
                nc.scalar.activation(out=attn_sb, in_=sc_ps, func=EXP,
                                     accum_out=sums[b][:, ch:ch + 1])
                at_ps = tps.tile([128, 512], BF16, tag="tp")
                for tt in range(2):
                    nc.tensor.transpose(
                        at_ps[:, tt * NH:(tt + 1) * NH],
                        attn_sb[:, tt * 128:(tt + 1) * 128],
                        ident[0:NH, 0:NH])
                at_sb = apool.tile([128, 2 * NH], BF16, tag="attnT")
                nc.vector.tensor_copy(out=at_sb, in_=at_ps[:, 0:2 * NH])
                # ctx accumulate: quadrant g holds D-chunks 2g, 2g+1
                for tt in range(2):
                    x_sb = x_tiles[(b, ch * 2 + tt)]
                    for g in range(4):
                        for jj in range(2):
                            nc.tensor.matmul(
                                ctx_ps[b][g * 32:(g + 1) * 32,
                                          jj * 512:(jj + 1) * 512],
                                at_sb[:, tt * NH:(tt + 1) * NH],
                                x_sb[:, (2 * g + jj) * 512:
                                     (2 * g + jj + 1) * 512],
                                start=(ch == 0 and tt == 0),
                                stop=(ch == NCH - 1 and tt == 1),
                                tile_position=(0, g * 32))

            cc2 = [None, None]

            def finalize_batch(b):
                ssum = smallp.tile([NH, 1], F32, tag="ssum")
                nc.vector.reduce_sum(out=ssum, in_=sums[b],
                                     axis=mybir.AxisListType.X)
                rsum = smallp.tile([128, 1], F32, tag="rsum")
                nc.vector.reciprocal(out=rsum[0:NH], in_=ssum)
                for g in range(1, 4):
                    nc.vector.tensor_copy(out=rsum[g * 32:(g + 1) * 32],
                                          in_=rsum[0:NH])
                ctx_sb = ctxsbp.tile([NH, DIM], BF16, tag="ctxsb")
                for j in range(8):
                    g = j // 2
                    src = ctx_ps[b][g * 32:(g + 1) * 32,
                                    (j % 2) * 512:(j % 2 + 1) * 512]
                    if j % 2 == 0:
                        nc.vector.tensor_scalar_mul(
                            ctx_sb[:, j * 512:(j + 1) * 512], src,
                            rsum[g * 32:(g + 1) * 32, 0:1])
                    else:
                        nc.scalar.mul(
                            out=ctx_sb[:, j * 512:(j + 1) * 512],
                            in_=src, mul=rsum[g * 32:(g + 1) * 32, 0:1])
                if debug:
                    nc.sync.dma_start(out=dbg_ctx[b * NH:(b + 1) * NH, :],
                                      in_=ctx_sb)
                d = nc.sync.dma_start(out=a2a2_in[b][:, :], in_=ctx_sb)
                cc = nc.gpsimd.collective_compute(
                    "AllToAll", mybir.AluOpType.bypass,
                    ins=[a2a2_in[b].opt()], outs=[a2a2_out[b].opt()],
                    replica_groups=rg)
                add_dep_helper(cc.ins, d.ins, reason="a2a2 input ready")
                cc2[b] = cc

            def project_batch(b):
                """wv projection + y partial for the heads gathered for b."""
                ctxg = gpool.tile([NC * HL, DIM], BF16, tag="gbuf",
                                  name=f"ctxg{b}")
                d = nc.sync.dma_start(out=ctxg, in_=a2a2_out[b][:, :])
                add_dep_helper(d.ins, cc2[b].ins, reason="a2a2 done")
                ctxgT = p3.tile([128, NDC * NC * HL], BF16, tag="ctxgT")
                for cg in range(4):
                    tp_t = tps.tile([128, 512], BF16, tag="tp")
                    tp = tp_t[:, 0:256]
                    for i in range(8):
                        c = cg * 8 + i
                        nc.tensor.transpose(
                            tp[:, i * 32:(i + 1) * 32],
                            ctxg[:, c * 128:(c + 1) * 128],
                            ident[0:32, 0:32])
                    cp = (nc.vector.tensor_copy if cg % 2 == 0
                          else act_copy)
                    cp(out=ctxgT[:, cg * 256:(cg + 1) * 256], in_=tp)
                op_f = scps.tile([64, 512], F32, tag="sc", name=f"op{b}")
                op_ps = op_f[0:NC * HL, 0:HD]
                for c in range(NDC):
                    nc.tensor.matmul(op_ps, ctxgT[:, c * 32:(c + 1) * 32],
                                     wvT[:, c * 128:(c + 1) * 128],
                                     start=(c == 0), stop=(c == NDC - 1))
                op_sb = p3.tile([NC * HL, HD], BF16, tag="op_sb")
                nc.vector.tensor_copy(out=op_sb, in_=op_ps)
                otp_t = tps.tile([128, 512], BF16, tag="tp")
                otp = otp_t[:, 0:32]
                nc.tensor.transpose(otp, op_sb, ident[0:32, 0:32])
                outT = p3.tile([128, NC * HL], BF16, tag="outT")
                nc.vector.tensor_copy(out=outT, in_=otp)
                oT = outT.rearrange("p (c h) -> p h c", h=HL)
                y_sb = p3.tile([NC, DIM], BF16, tag="ysb")
                for jc in range(8):
                    y_f = scps.tile([64, 512], F32, tag="sc",
                                    name=f"y{b}_{jc}")
                    y_ps = y_f[0:NC, :]
                    for m in range(HL):
                        nc.tensor.matmul(
                            y_ps, oT[:, m, :],
                            woT[:, m * DIM + jc * 512:
                                m * DIM + (jc + 1) * 512],
                            start=(m == 0), stop=(m == HL - 1))
                    cp = (nc.vector.tensor_copy if jc % 2 == 0
                          else act_copy)
                    cp(out=y_sb[:, jc * 512:(jc + 1) * 512], in_=y_ps)
                nc.gpsimd.dma_start(out=y[b * NC:(b + 1) * NC, :], in_=y_sb)

            # ---------------- the flat pipeline over 16 chunks
            xtts = {}
            xtts[0] = transpose_chunk(0)
            xtts[1] = transpose_chunk(1)
            wv_transposes()
            for cg in range(8):
                wo_transposes(cg)
            qnat_unpack(0)
            for k in range(NK):
                issue_x(k + 3)
                attend_chunk(k, xtts.pop(k))
                if k + 2 < NK:
                    xtts[k + 2] = transpose_chunk(k + 2)
                if k == 0:
                    qnat_unpack(1)
                if k == NCH - 1:
                    finalize_batch(0)
                if k == NCH + 3:
                    project_batch(0)
            finalize_batch(1)
            project_batch(1)

    nc.finalize()
    return nc


_PROGRAM_CACHE = {}


def kernel(x_pre, wq, wk, wv, wo, _trace=False, _tmpdir=None, _debug=False):
    x_pre = np.ascontiguousarray(np.asarray(x_pre, dtype=np.float32))
    wq = np.asarray(wq, dtype=np.float32)
    wk = np.asarray(wk, dtype=np.float32)
    wv = np.asarray(wv, dtype=np.float32)
    wo = np.asarray(wo, dtype=np.float32)

    key = ("dbg" if _debug else "nc")
    if key not in _PROGRAM_CACHE:
        _PROGRAM_CACHE[key] = build_program(debug=_debug)
    nc = _PROGRAM_CACHE[key]

    xl = np.ascontiguousarray(x_pre[:, -1, :])
    in_maps = []
    for i in range(NC):
        in_maps.append({
            "xp": np.ascontiguousarray(x_pre[2 * i:2 * i + 2]),
            "xl": xl,
            "wq": np.ascontiguousarray(wq[512 * i:512 * (i + 1), :]),
            "wk": np.ascontiguousarray(wk[128 * i:128 * (i + 1), :]),
            "wv": np.ascontiguousarray(wv[128 * i:128 * (i + 1), :]),
            "wo": np.ascontiguousarray(wo[:, 512 * i:512 * (i + 1)]),
        })

    kwargs = {}
    if _trace:
        kwargs = dict(trace=True, trace_cores=[0])
    if _tmpdir is not None:
        kwargs["tmpdir"] = _tmpdir
    res = run_bass_kernel_spmd(nc, in_maps, core_ids=list(range(NC)), **kwargs)
    y = np.zeros((BSZ, DIM), np.float32)
    for i in range(NC):
        yi = res.results[i]["y"]
        # kernel y row r = 8*b_loc + c  ->  global batch 2*c + b_loc
        y[0::2] += yi[0:NC]
        y[1::2] += yi[NC:2 * NC]
    if _debug:
        _PROGRAM_CACHE["dbg_res"] = res
    if _trace:
        print("HW exec time:", res.exec_time_ns, "ns")
    return y.reshape(BSZ, 1, DIM)


# revision 21
# speedup vs baseline: 2.7428x; 1.1515x over previous
"""Bass/Trainium2 kernel for GQA decode attention (fused K-projection form).

Reference computation (per problem spec):
  x = x_pre[:, -1, :]                               # [16, 4096]
  xq = (x @ wq.T) -> [b, 32, 128]
  qt[b,h,:] = xq[b,h,:] @ wk[kv(h)*128:+128, :]     # [b, 32, 4096]
  scores = qt . x_pre / sqrt(128)                   # [b, 32, 2048]
  attn = softmax_t(scores)
  ctx[b,h,:] = sum_t attn[b,h,t] * x_pre[b,t,:]     # [b, 32, 4096]  (lazy-V)
  out[b,h,d] = sum_D ctx[b,h,D] * wv[kv(h)*128+d,D] # [b, 32, 128]
  y = out.flat @ wo.T                               # [16, 4096]

Sharding (8 cores): batch-parallel attention (2 batches/core) +
head-parallel projections (4 heads = 1 kv head/core).  All on-chip compute
in bf16 (fp32 PSUM accumulation); HBM loads cast f32->bf16 during the DMA
(SWDGE).  Two AllToAlls redistribute qt (head->batch) and ctx
(batch->head), each split per local batch so the exchange overlaps
compute.  wo is column-sharded; host sums the 8 partial y outputs.

The per-core program is laid out as a flat software pipeline over the 16
(batch, 256-token-chunk) attention steps so every engine's in-order
instruction stream matches expected data-arrival order (engine wait
queues are only 4 deep - a burst of >4 not-yet-satisfiable instructions
stalls the whole engine stream).
"""

import math
from contextlib import ExitStack

import numpy as np

import concourse.mybir as mybir
import concourse.tile as tile
from concourse import bacc
from concourse.bass_utils import run_bass_kernel_spmd
from concourse.masks import make_identity
from concourse.tile import add_dep_helper

F32 = mybir.dt.float32
BF16 = mybir.dt.bfloat16
NC = 8
BSZ = 16
SEQ = 2048
DIM = 4096
NH = 32
HD = 128
NKV = 8
B_LOC = 2        # batches per core
HL = 4           # local heads per core
NT = SEQ // 128  # 16 t-tiles per batch
NCH = 8          # 256-token chunks per batch
NK = B_LOC * NCH # 16 flat chunks
NDC = DIM // 128 # 32 D-chunks
SCALE = 1.0 / math.sqrt(HD)
EXP = mybir.ActivationFunctionType.Exp


def build_program(debug=False):
    nc = bacc.Bacc("TRN2", target_bir_lowering=False, debug=False)

    def act_copy(out, in_):
        return nc.scalar.mul(out=out, in_=in_, mul=1.0)

    xp = nc.dram_tensor("xp", [B_LOC, SEQ, DIM], F32, kind="ExternalInput")
    xl = nc.dram_tensor("xl", [BSZ, DIM], F32, kind="ExternalInput")
    wq = nc.dram_tensor("wq", [HL * HD, DIM], F32, kind="ExternalInput")
    wk = nc.dram_tensor("wk", [HD, DIM], F32, kind="ExternalInput")
    wv = nc.dram_tensor("wv", [HD, DIM], F32, kind="ExternalInput")
    wo = nc.dram_tensor("wo", [DIM, HL * HD], F32, kind="ExternalInput")
    y = nc.dram_tensor("y", [BSZ, DIM], F32, kind="ExternalOutput")
    if debug:
        dbg_qt = nc.dram_tensor("dbg_qt", [B_LOC * NH, DIM], BF16,
                                kind="ExternalOutput")
        dbg_ctx = nc.dram_tensor("dbg_ctx", [B_LOC * NH, DIM], BF16,
                                 kind="ExternalOutput")

    rg = [list(range(NC))]

    with tile.TileContext(nc) as tc:
        with ExitStack() as es:
            pers = es.enter_context(tc.tile_pool(name="persist", bufs=1))
            dram = es.enter_context(
                tc.tile_pool(name="dram", bufs=1, space="DRAM"))
            p1 = es.enter_context(tc.tile_pool(name="p1", bufs=2))
            p1w = es.enter_context(tc.tile_pool(name="p1w", bufs=1))
            gpool = es.enter_context(tc.tile_pool(name="gpool", bufs=1))
            pw = es.enter_context(tc.tile_pool(name="pw", bufs=2))
            xpool = es.enter_context(tc.tile_pool(name="xpool", bufs=6))
            xtpool = es.enter_context(tc.tile_pool(name="xtpool", bufs=3))
            apool = es.enter_context(tc.tile_pool(name="attn", bufs=2))
            smallp = es.enter_context(tc.tile_pool(name="small", bufs=2))
            ctxsbp = es.enter_context(tc.tile_pool(name="ctx_sbp", bufs=1))
            p3 = es.enter_context(tc.tile_pool(name="p3", bufs=1))
            # PSUM budget (8 banks): tp x3 + sc x3 + ctx(2 banks)
            tps = es.enter_context(
                tc.tile_pool(name="tps", bufs=3, space="PSUM"))
            scps = es.enter_context(
                tc.tile_pool(name="scps", bufs=3, space="PSUM"))
            ctxps = es.enter_context(
                tc.tile_pool(name="ctxps", bufs=1, space="PSUM"))

            ident = pers.tile([128, 128], BF16)
            make_identity(nc, ident)

            # DRAM exchange buffers (bf16)
            a2a1_in = [dram.tile([NC * HL, DIM], BF16, name=f"a1i{b}")
                       for b in range(B_LOC)]
            a2a1_out = [dram.tile([NC * HL, DIM], BF16, name=f"a1o{b}")
                        for b in range(B_LOC)]
            a2a2_in = [dram.tile([NC * HL, DIM], BF16, name=f"a2i{b}")
                       for b in range(B_LOC)]
            a2a2_out = [dram.tile([NC * HL, DIM], BF16, name=f"a2o{b}")
                        for b in range(B_LOC)]

            qT = [pers.tile([128, NDC * NH], BF16, name=f"qT{b}")
                  for b in range(B_LOC)]
            wvT = pers.tile([128, DIM], BF16)
            woT = pers.tile([128, HL * DIM], BF16)

            # ---------------- x-tile DMA prefetch machinery
            x_tiles = {}

            def issue_x_tile(b, t):
                x_sb = xpool.tile([128, DIM], BF16, tag="x",
                                  name=f"x{b}_{t}")
                nc.gpsimd.dma_start(
                    out=x_sb, in_=xp[b, t * 128:(t + 1) * 128, :])
                x_tiles[(b, t)] = x_sb

            def issue_x(k):
                """DMA the two 128-token tiles of flat chunk k."""
                if k >= NK:
                    return
                b, ch = divmod(k, NCH)
                for tt in range(2):
                    issue_x_tile(b, ch * 2 + tt)

            # ---------------- Phase 1 + interleaved DMA prologue
            xl_sb = p1.tile([BSZ, DIM], BF16, tag="wqm", bufs=1,
                            name="xl_sb")
            nc.gpsimd.dma_start(out=xl_sb, in_=xl[:, :])
            issue_x_tile(0, 0)
            issue_x_tile(0, 1)

            # xlT: [128 D x 16 b] per D-chunk
            xlT = p1w.tile([128, NDC * BSZ], BF16, name="xlT")
            for q in range(4):
                tp = tps.tile([128, 512], BF16, tag="tp")
                for i in range(8):
                    c = q * 8 + i
                    nc.tensor.transpose(
                        tp[:, i * BSZ:(i + 1) * BSZ],
                        xl_sb[:, c * 128:(c + 1) * 128],
                        ident[0:BSZ, 0:BSZ])
                nc.vector.tensor_copy(
                    out=xlT[:, q * 8 * BSZ:(q + 1) * 8 * BSZ],
                    in_=tp[:, 0:8 * BSZ])

            # xq = x @ wq_slice.T, one 128-row block of wq at a time
            xq_f = ctxps.tile([128, 1024], F32, tag="ctx", name="xq_f")
            xq_ps = xq_f[0:BSZ, 0:HL * HD]
            wk_sb = p1w.tile([HD, DIM], BF16, name="wk_sb")
            for m in range(HL):
                wq_m = p1.tile([128, DIM], BF16, tag="wqm", bufs=1)
                nc.gpsimd.dma_start(out=wq_m,
                                    in_=wq[m * 128:(m + 1) * 128, :])
                if m == 0:
                    nc.gpsimd.dma_start(out=wk_sb, in_=wk[:, :])
                elif m == 1:
                    issue_x_tile(0, 2)
                elif m == 2:
                    issue_x_tile(0, 3)
                wqT_m = p1.tile([128, DIM], BF16, tag="wqTm")
                for cg in range(8):
                    tp = tps.tile([128, 512], BF16, tag="tp")
                    for i in range(4):
                        c = cg * 4 + i
                        nc.tensor.transpose(tp[:, i * 128:(i + 1) * 128],
                                            wq_m[:, c * 128:(c + 1) * 128],
                                            ident)
                    cp = (nc.vector.tensor_copy if cg % 2 == 0
                          else act_copy)
                    cp(out=wqT_m[:, cg * 512:(cg + 1) * 512], in_=tp)
                for c in range(NDC):
                    nc.tensor.matmul(xq_ps[:, m * 128:(m + 1) * 128],
                                     xlT[:, c * BSZ:(c + 1) * BSZ],
                                     wqT_m[:, c * 128:(c + 1) * 128],
                                     start=(c == 0), stop=(c == NDC - 1))
            # scale by 1/sqrt(HD) here (folds the score scaling)
            xq_sb = p1w.tile([BSZ, HL * HD], BF16, name="xq_sb")
            nc.scalar.mul(out=xq_sb, in_=xq_ps, mul=SCALE)

            # xqT columns ordered (bl, r2, m)  [source cols (m, r2, bl)]
            xqT = p1w.tile([128, HL * BSZ], BF16, name="xqT")
            tpq_t = tps.tile([128, 512], BF16, tag="tp")
            tpq = tpq_t[:, 0:HL * BSZ]
            for m in range(HL):
                nc.tensor.transpose(
                    tpq[:, m * BSZ:(m + 1) * BSZ],
                    xq_sb[:, m * 128:(m + 1) * 128],
                    ident[0:BSZ, 0:BSZ])
            nc.vector.tensor_copy(
                out=xqT.rearrange("p (bl r2 m) -> p m r2 bl",
                                  bl=B_LOC, r2=NC, m=HL),
                in_=tpq.rearrange("p (m r2 bl) -> p m r2 bl",
                                  m=HL, r2=NC, bl=B_LOC))

            # qt rows (bl, r2, m): [64, 4096] = xqT.T @ wk (natural layout!)
            qt_sb = gpool.tile([BSZ * HL, DIM], BF16, tag="gbuf",
                                name="qt_sb")
            for j in range(8):
                q_f = scps.tile([64, 512], F32, tag="sc", name=f"q_f{j}")
                nc.tensor.matmul(q_f, xqT,
                                 wk_sb[:, j * 512:(j + 1) * 512],
                                 start=True, stop=True)
                cp = (nc.vector.tensor_copy if j % 2 == 0
                      else act_copy)
                cp(out=qt_sb[:, j * 512:(j + 1) * 512], in_=q_f)

            # stage per b_loc (contiguous row slices) + collectives
            cc1 = []
            for bl in range(B_LOC):
                d = nc.sync.dma_start(out=a2a1_in[bl][:, :],
                                      in_=qt_sb[bl * 32:(bl + 1) * 32, :])
                cc = nc.gpsimd.collective_compute(
                    "AllToAll", mybir.AluOpType.bypass,
                    ins=[a2a1_in[bl].opt()], outs=[a2a1_out[bl].opt()],
                    replica_groups=rg)
                add_dep_helper(cc.ins, d.ins, reason="a2a1 input ready")
                cc1.append(cc)

            # more DMA prologue on the Pool queue
            wv_sb = p1.tile([128, DIM], BF16, tag="wqm", bufs=1,
                            name="wv_sb")
            nc.gpsimd.dma_start(out=wv_sb, in_=wv[:, :])
            issue_x(2)
            wo_list = []
            for cg in range(8):
                wo_cg = pw.tile([128, 4 * HL * HD], BF16, tag="wocg",
                                name=f"wo{cg}", bufs=1)
                nc.gpsimd.dma_start(
                    out=wo_cg,
                    in_=wo[cg * 512:(cg + 1) * 512, :].rearrange(
                        "(c p) j -> p c j", p=128))
                wo_list.append(wo_cg)

            # ---------------- pipeline helper stages
            def make_tp_groups(k):
                """Allocate chunk k's xtt slot; return it plus 8 thunks that
                each PE-transpose one [128,1024] group and copy it out."""
                b, ch = divmod(k, NCH)
                xtt = xtpool.tile([128, NDC * 256], BF16, tag="xt",
                                  name=f"xt{k}")
                xv = xtt.rearrange("p (c t) -> p c t", c=NDC)
                thunks = []
                for tt in range(2):
                    for cg in range(4):
                        def thunk(tt=tt, cg=cg, b=b, ch=ch, xv=xv,
                                  g=2 * tt + cg):
                            x_sb = x_tiles[(b, ch * 2 + tt)]
                            tp = tps.tile([128, 1024], BF16, tag="tp")
                            for i in range(8):
                                c = cg * 8 + i
                                nc.tensor.transpose(
                                    tp[:, i * 128:(i + 1) * 128],
                                    x_sb[:, c * 128:(c + 1) * 128],
                                    ident)
                            cp = (nc.vector.tensor_copy
                                  if g not in (1, 4, 6) else act_copy)
                            cp(out=xv[:, cg * 8:(cg + 1) * 8,
                                      tt * 128:(tt + 1) * 128],
                               in_=tp.rearrange("p (a t) -> p a t", a=8))
                        thunks.append(thunk)
                return xtt, thunks

            def wv_transposes():
                for cg in range(8):
                    tp = tps.tile([128, 512], BF16, tag="tp")
                    for i in range(4):
                        c = cg * 4 + i
                        nc.tensor.transpose(tp[:, i * 128:(i + 1) * 128],
                                            wv_sb[:, c * 128:(c + 1) * 128],
                                            ident)
                    cp = (nc.vector.tensor_copy if cg % 2 == 0
                          else act_copy)
                    cp(out=wvT[:, cg * 512:(cg + 1) * 512], in_=tp)

            def wo_transposes(cg):
                wo_cg = wo_list[cg]
                for m in range(HL):
                    tp = tps.tile([128, 512], BF16, tag="tp")
                    for i in range(4):
                        nc.tensor.transpose(
                            tp[:, i * 128:(i + 1) * 128],
                            wo_cg[:, i * 512 + m * 128:
                                  i * 512 + (m + 1) * 128],
                            ident)
                    cp = (nc.vector.tensor_copy if m % 2 == 0
                          else act_copy)
                    cp(out=woT[:, m * DIM + cg * 512:
                               m * DIM + (cg + 1) * 512],
                       in_=tp)

            def qnat_unpack(b):
                qnat = gpool.tile([NH, DIM], BF16, tag="gbuf",
                                  name=f"qnat{b}")
                d = nc.sync.dma_start(out=qnat, in_=a2a1_out[b][:, :])
                add_dep_helper(d.ins, cc1[b].ins, reason="a2a1 done")
                if debug:
                    nc.sync.dma_start(out=dbg_qt[b * NH:(b + 1) * NH, :],
                                      in_=qnat)
                for cg in range(8):
                    tp_t = tps.tile([128, 512], BF16, tag="tp")
                    tp = tp_t[:, 0:4 * NH]
                    for i in range(4):
                        c = cg * 4 + i
                        nc.tensor.transpose(
                            tp[:, i * NH:(i + 1) * NH],
                            qnat[:, c * 128:(c + 1) * 128],
                            ident[0:NH, 0:NH])
                    cp = (nc.vector.tensor_copy if cg % 2 == 0
                          else act_copy)
                    cp(out=qT[b][:, cg * 4 * NH:(cg + 1) * 4 * NH], in_=tp)

            ctx_ps = [None, None]
            sums = [None, None]

            def attend_chunk(k, xtt, pending=()):
                pending = list(pending)
                b, ch = divmod(k, NCH)
                if ch == 0:
                    ctx_ps[b] = ctxps.tile([128, 1024], F32, tag="ctx",
                                           name=f"ctx{b}")
                    sums[b] = smallp.tile([NH, NCH], F32, tag="sums",
                                          name=f"sums{b}")
                sc_ps_t = scps.tile([64, 512], F32, tag="sc",
                                    name=f"sc{k}")
                sc_ps = sc_ps_t[0:NH, 0:256]
                for c in range(NDC):
                    nc.tensor.matmul(
                        sc_ps, qT[b][:, c * NH:(c + 1) * NH],
                        xtt[:, c * 256:(c + 1) * 256],
                        start=(c == 0), stop=(c == NDC - 1))
                attn_sb = apool.tile([NH, 256], BF16, tag="attn")
                nc.scalar.activation(out=attn_sb, in_=sc_ps, func=EXP,
                                     accum_out=sums[b][:, ch:ch + 1])
                at_ps = tps.tile([128, 512], BF16, tag="tp")
                for tt in range(2):
                    nc.tensor.transpose(
                        at_ps[:, tt * NH:(tt + 1) * NH],
                        attn_sb[:, tt * 128:(tt + 1) * 128],
                        ident[0:NH, 0:NH])
                at_sb = apool.tile([128, 2 * NH], BF16, tag="attnT")
                nc.vector.tensor_copy(out=at_sb, in_=at_ps[:, 0:2 * NH])
                # ctx accumulate: quadrant g holds D-chunks 2g, 2g+1
                for tt in range(2):
                    x_sb = x_tiles[(b, ch * 2 + tt)]
                    for g in range(4):
                        for jj in range(2):
                            nc.tensor.matmul(
                                ctx_ps[b][g * 32:(g + 1) * 32,
                                          jj * 512:(jj + 1) * 512],
                                at_sb[:, tt * NH:(tt + 1) * NH],
                                x_sb[:, (2 * g + jj) * 512:
                                     (2 * g + jj + 1) * 512],
                                start=(ch == 0 and tt == 0),
                                stop=(ch == NCH - 1 and tt == 1),
                                tile_position=(0, g * 32))
                        if pending:
                            pending.pop(0)()
                while pending:
                    pending.pop(0)()

            cc2 = [None, None]

            def finalize_batch(b):
                ssum = smallp.tile([NH, 1], F32, tag="ssum")
                nc.vector.reduce_sum(out=ssum, in_=sums[b],
                                     axis=mybir.AxisListType.X)
                rsum = smallp.tile([128, 1], F32, tag="rsum")
                nc.vector.reciprocal(out=rsum[0:NH], in_=ssum)
                for g in range(1, 4):
                    nc.vector.tensor_copy(out=rsum[g * 32:(g + 1) * 32],
                                          in_=rsum[0:NH])
                ctx_sb = ctxsbp.tile([NH, DIM], BF16, tag="ctxsb")
                for j in range(8):
                    g = j // 2
                    src = ctx_ps[b][g * 32:(g + 1) * 32,
                                    (j % 2) * 512:(j % 2 + 1) * 512]
                    if j % 2 == 0:
                        nc.vector.tensor_scalar_mul(
                            ctx_sb[:, j * 512:(j + 1) * 512], src,
                            rsum[g * 32:(g + 1) * 32, 0:1])
                    else:
                        nc.scalar.mul(
                            out=ctx_sb[:, j * 512:(j + 1) * 512],
                            in_=src, mul=rsum[g * 32:(g + 1) * 32, 0:1])
                if debug:
                    nc.sync.dma_start(out=dbg_ctx[b * NH:(b + 1) * NH, :],
                                      in_=ctx_sb)
                d = nc.sync.dma_start(out=a2a2_in[b][:, :], in_=ctx_sb)
                cc = nc.gpsimd.collective_compute(
                    "AllToAll", mybir.AluOpType.bypass,
                    ins=[a2a2_in[b].opt()], outs=[a2a2_out[b].opt()],
                    replica_groups=rg)
                add_dep_helper(cc.ins, d.ins, reason="a2a2 input ready")
                cc2[b] = cc

            def project_batch(b):
                """wv projection + y partial for the heads gathered for b."""
                ctxg = gpool.tile([NC * HL, DIM], BF16, tag="gbuf",
                                  name=f"ctxg{b}")
                d = nc.sync.dma_start(out=ctxg, in_=a2a2_out[b][:, :])
                add_dep_helper(d.ins, cc2[b].ins, reason="a2a2 done")
                ctxgT = p3.tile([128, NDC * NC * HL], BF16, tag="ctxgT")
                for cg in range(4):
                    tp_t = tps.tile([128, 512], BF16, tag="tp")
                    tp = tp_t[:, 0:256]
                    for i in range(8):
                        c = cg * 8 + i
                        nc.tensor.transpose(
                            tp[:, i * 32:(i + 1) * 32],
                            ctxg[:, c * 128:(c + 1) * 128],
                            ident[0:32, 0:32])
                    cp = (nc.vector.tensor_copy if cg % 2 == 0
                          else act_copy)
                    cp(out=ctxgT[:, cg * 256:(cg + 1) * 256], in_=tp)
                op_f = scps.tile([64, 512], F32, tag="sc", name=f"op{b}")
                op_ps = op_f[0:NC * HL, 0:HD]
                for c in range(NDC):
                    nc.tensor.matmul(op_ps, ctxgT[:, c * 32:(c + 1) * 32],
                                     wvT[:, c * 128:(c + 1) * 128],
                                     start=(c == 0), stop=(c == NDC - 1))
                op_sb = p3.tile([NC * HL, HD], BF16, tag="op_sb")
                nc.vector.tensor_copy(out=op_sb, in_=op_ps)
                otp_t = tps.tile([128, 512], BF16, tag="tp")
                otp = otp_t[:, 0:32]
                nc.tensor.transpose(otp, op_sb, ident[0:32, 0:32])
                outT = p3.tile([128, NC * HL], BF16, tag="outT")
                nc.vector.tensor_copy(out=outT, in_=otp)
                oT = outT.rearrange("p (c h) -> p h c", h=HL)
                y_sb = gpool.tile([NC, DIM], BF16, tag="gbuf",
                                  name=f"ysb{b}")
                for jc in range(8):
                    y_f = scps.tile([64, 512], F32, tag="sc",
                                    name=f"y{b}_{jc}")
                    y_ps = y_f[0:NC, :]
                    for m in range(HL):
                        nc.tensor.matmul(
                            y_ps, oT[:, m, :],
                            woT[:, m * DIM + jc * 512:
                                m * DIM + (jc + 1) * 512],
                            start=(m == 0), stop=(m == HL - 1))
                    cp = (nc.vector.tensor_copy if jc % 2 == 0
                          else act_copy)
                    cp(out=y_sb[:, jc * 512:(jc + 1) * 512], in_=y_ps)
                nc.gpsimd.dma_start(out=y[b * NC:(b + 1) * NC, :], in_=y_sb)

            # ---------------- the flat pipeline over 16 chunks
            xtt0, th0 = make_tp_groups(0)
            for t in th0:
                t()
            xtt1, th1 = make_tp_groups(1)
            for t in th1:
                t()
            wv_transposes()
            for cg in range(8):
                wo_transposes(cg)
            qnat_unpack(0)
            xtts = {0: xtt0, 1: xtt1}
            thunks = {}
            for k in range(NK):
                issue_x(k + 3)
                if k + 2 < NK:
                    xtts[k + 2], thunks[k + 2] = make_tp_groups(k + 2)
                attend_chunk(k, xtts.pop(k), thunks.pop(k + 2, ()))
                if k == 0:
                    qnat_unpack(1)
                if k == NCH - 1:
                    finalize_batch(0)
                if k == NCH + 3:
                    project_batch(0)
            finalize_batch(1)
            project_batch(1)

    nc.finalize()
    return nc


_PROGRAM_CACHE = {}


def kernel(x_pre, wq, wk, wv, wo, _trace=False, _tmpdir=None, _debug=False):
    x_pre = np.ascontiguousarray(np.asarray(x_pre, dtype=np.float32))
    wq = np.asarray(wq, dtype=np.float32)
    wk = np.asarray(wk, dtype=np.float32)
    wv = np.asarray(wv, dtype=np.float32)
    wo = np.asarray(wo, dtype=np.float32)

    key = ("dbg" if _debug else "nc")
    if key not in _PROGRAM_CACHE:
        _PROGRAM_CACHE[key] = build_program(debug=_debug)
    nc = _PROGRAM_CACHE[key]

    xl = np.ascontiguousarray(x_pre[:, -1, :])
    in_maps = []
    for i in range(NC):
        in_maps.append({
            "xp": np.ascontiguousarray(x_pre[2 * i:2 * i + 2]),
            "xl": xl,
            "wq": np.ascontiguousarray(wq[512 * i:512 * (i + 1), :]),
            "wk": np.ascontiguousarray(wk[128 * i:128 * (i + 1), :]),
            "wv": np.ascontiguousarray(wv[128 * i:128 * (i + 1), :]),
            "wo": np.ascontiguousarray(wo[:, 512 * i:512 * (i + 1)]),
        })

    kwargs = {}
    if _trace:
        kwargs = dict(trace=True, trace_cores=[0])
    if _tmpdir is not None:
        kwargs["tmpdir"] = _tmpdir
    res = run_bass_kernel_spmd(nc, in_maps, core_ids=list(range(NC)), **kwargs)
    y = np.zeros((BSZ, DIM), np.float32)
    for i in range(NC):
        yi = res.results[i]["y"]
        # kernel y row r = 8*b_loc + c  ->  global batch 2*c + b_loc
        y[0::2] += yi[0:NC]
        y[1::2] += yi[NC:2 * NC]
    if _debug:
        _PROGRAM_CACHE["dbg_res"] = res
    if _trace:
        print("HW exec time:", res.exec_time_ns, "ns")
    return y.reshape(BSZ, 1, DIM)
